# revision 1
# baseline (speedup 1.0000x reference)
"""Trainium2 Bass kernel for the gated two-path (semantic+RoPE-geometric) causal
attention layer.  8-core sharding: 2 batches x 4 head-groups (4 heads each).

Reference computation (B=2, S=2048, D_MODEL=2048, H=16, DS=DG=64, DV=128):
  qs=x@wq_sem, ks=x@wk_sem, qg=rope(x@wq_geo), kg=rope(x@wk_geo), v=x@wv
  scores = g*qs.ks/8 + (1-g)*qg.kg/8 ; causal softmax ; out=(attn@v)@wo

Per-core kernel strategy (all matmuls float32r, transposed dataflow):
  - host folds sigmoid(gate)/sqrt(d) scales into wq and concatenates
    [sem|geo] per head so each head's QK^T is one K=128 contraction
  - weights are pre-tiled on the host into the exact SBUF layouts so every
    DMA moves >=2KB-contiguous runs
  - projections: qcatT/kcatT [128, S] per head via lhsT=weight tiles,
    rhs=xT chunks; v in natural [token, dv] layout via lhsT=xT tiles;
    rope applied per 512-token slice right after each eviction
  - scores^T [k,128 x q,512] per k-tile; causal mask added via an
    identity-lhsT matmul of a sliding window into one [128,896] staircase
  - exp on ScalarE (no max subtraction; |scores| <~ 8 << 88), AV + ones-matmul
    denominators accumulate in PSUM, gpsimd broadcast + fast reciprocal
  - output projection accumulates over the 4 heads -> outT [D_MODEL, S];
    host transposes and sums the 4 head-group partials per batch
"""

import os
import sys

sys.path.insert(0, "/opt/trn_rl_repo")

import ml_dtypes
import numpy as np

import concourse.mybir as mybir
import concourse.tile as tile
from concourse import bacc
from concourse.bass_utils import run_bass_kernel_spmd

F32 = mybir.dt.float32
F32R = mybir.dt.float32r
F16 = mybir.dt.float16
BF16 = mybir.dt.bfloat16

# bf16 projections: halves projection DMA traffic; rel err ~3.3e-3 vs ~3.4e-4
PROJ_BF16 = os.environ.get("KERNEL_PROJ_BF16", "0") == "1"
PROJ_DT = BF16 if PROJ_BF16 else F32R
PROJ_NP = ml_dtypes.bfloat16 if PROJ_BF16 else np.float32
# apply the causal staircase on DVE (tensor_add into PSUM) instead of an
# identity-lhsT matmul on PE
MASK_DVE = os.environ.get("KERNEL_MASK_DVE", "0") == "1"

B, S, DM = 2, 2048, 2048
H, DS, DG, DV = 16, 64, 64, 128
HPC = 4                      # heads per core
NCORES = 8
DH = DS + DG                 # 128, concat [sem|geo] per head
NKT = S // 128               # 16 key tiles
NQB = S // 512               # 4 query blocks
NTCH = 4                     # token chunks of 512
NDMK = DM // 128             # 16 contraction tiles
MASK_VAL = -10000.0

_CACHED = {}


def _build(repeat=1):
    nc = bacc.Bacc("TRN2", target_bir_lowering=False, debug=False,
                   num_devices=NCORES)

    xT = nc.dram_tensor("xT", [DM, S], PROJ_DT, kind="ExternalInput").ap()
    # pre-tiled weights (see _host_prep for layouts)
    wqk_d = nc.dram_tensor("wqk", [2 * HPC, 128, NDMK, 128], PROJ_DT,
                           kind="ExternalInput").ap()
    wv_d = nc.dram_tensor("wv", [128, NDMK, 512], PROJ_DT,
                          kind="ExternalInput").ap()
    wo_d = nc.dram_tensor("wo", [NDMK, 128, HPC, 128], F32R,
                          kind="ExternalInput").ap()
    cos2_d = nc.dram_tensor("cos2", [DG, S], F16, kind="ExternalInput").ap()
    sins_d = nc.dram_tensor("sins", [DG, S], F16, kind="ExternalInput").ap()
    masks_d = nc.dram_tensor("masks", [128, 896], F32R,
                             kind="ExternalInput").ap()
    ident_d = nc.dram_tensor("ident", [128, 128], F32R,
                             kind="ExternalInput").ap()
    ones_d = nc.dram_tensor("ones", [128, 1], F32R, kind="ExternalInput").ap()
    out_d = nc.dram_tensor("out", [DM, S], F32, kind="ExternalOutput").ap()

    Exp = mybir.ActivationFunctionType.Exp

    with tile.TileContext(nc) as tc:
      for _rep in range(repeat):
        with tc.tile_pool(name="consts", bufs=1) as cpool, \
             tc.tile_pool(name="attn_out", bufs=1) as aopool:
            ident = cpool.tile([128, 128], F32R)
            ones = cpool.tile([128, 1], F32R)
            # trig tables at base partition 64 so two-input DVE rope ops
            # share their operands' base partition
            trig = cpool.tile([128, 2, S], F16)
            masksB = cpool.tile([128, 896], F32R)
            nc.sync.dma_start(out=ident[:], in_=ident_d[:, :])
            nc.sync.dma_start(out=ones[:], in_=ones_d[:, :])
            nc.sync.dma_start(out=trig[64:128, 0, :], in_=cos2_d[:, :])
            nc.sync.dma_start(out=trig[64:128, 1, :], in_=sins_d[:, :])
            nc.sync.dma_start(out=masksB[:], in_=masks_d[:, :])
            attn_outT = aopool.tile([128, HPC, S], F32R)

            with tc.tile_pool(name="persist", bufs=1) as ppool:
                qcatT = ppool.tile([128, HPC, S], F32R)
                kcatT = ppool.tile([128, HPC, S], F32R)
                v_sb = ppool.tile([128, NKT, 512], F32R)

                # ---------------- phase 1: projections (+ rope fused) ------
                with tc.tile_pool(name="xt", bufs=NDMK + 2) as xtp, \
                     tc.tile_pool(name="wcol", bufs=6) as wcp, \
                     tc.tile_pool(name="wvst", bufs=6) as wvp, \
                     tc.tile_pool(name="rot", bufs=2) as rpool, \
                     tc.tile_pool(name="psA", bufs=4, space="PSUM") as psA:

                    def load_wq(fb, qq):
                        wc_q = wcp.tile([128, 4, 128], PROJ_DT, tag="wc")
                        nc.sync.dma_start(
                            out=wc_q[:],
                            in_=wqk_d[fb, :, 4 * qq:4 * qq + 4, :])
                        return wc_q

                    for tch in range(NTCH):
                        ts_ = slice(tch * 512, tch * 512 + 512)
                        # first feature column's weights before the x chunks
                        # so the first PSUM chain can start immediately
                        wcs = {(0, 0): load_wq(0, 0)}
                        xts = []
                        for dmk in range(NDMK):
                            xt_t = xtp.tile([128, 512], PROJ_DT, tag="xt")
                            nc.sync.dma_start(
                                out=xt_t[:],
                                in_=xT[dmk * 128:dmk * 128 + 128, ts_])
                            xts.append(xt_t)
                        # qcat / kcat columns: 8 feature tiles of 128
                        for fb in range(2 * HPC):
                            h = fb % HPC
                            for qq in range(4):
                                if (fb, qq) not in wcs:
                                    wcs[(fb, qq)] = load_wq(fb, qq)
                            ps_t = psA.tile([128, 512], F32, tag="ps")
                            for dmk in range(NDMK):
                                nc.tensor.matmul(
                                    ps_t[:],
                                    wcs[(fb, dmk // 4)][:, dmk % 4, :],
                                    xts[dmk][:],
                                    start=(dmk == 0), stop=(dmk == NDMK - 1))
                            X = qcatT if fb < HPC else kcatT
                            nc.scalar.copy(X[:, h, ts_], ps_t[:])
                            # rope this 512-token slice of the geo half
                            rot = rpool.tile([128, 512], F32R, tag="rot")
                            nc.gpsimd.tensor_copy(rot[64:96, :],
                                                  X[96:128, h, ts_])
                            nc.gpsimd.tensor_copy(rot[96:128, :],
                                                  X[64:96, h, ts_])
                            nc.vector.tensor_mul(rot[64:128, :],
                                                 rot[64:128, :],
                                                 trig[64:128, 1, ts_])
                            nc.vector.tensor_mul(X[64:128, h, ts_],
                                                 X[64:128, h, ts_],
                                                 trig[64:128, 0, ts_])
                            nc.vector.tensor_add(X[64:128, h, ts_],
                                                 X[64:128, h, ts_],
                                                 rot[64:128, :])
                        # v: natural layout, 4 token sub-tiles
                        wvt = []
                        for dmk in range(NDMK):
                            wv_t = wvp.tile([128, 512], PROJ_DT, tag="wv")
                            nc.sync.dma_start(out=wv_t[:], in_=wv_d[:, dmk, :])
                            wvt.append(wv_t)
                        for tsub in range(4):
                            tt = tch * 4 + tsub
                            ps_v = psA.tile([128, 512], F32, tag="ps")
                            for dmk in range(NDMK):
                                nc.tensor.matmul(
                                    ps_v[:],
                                    xts[dmk][:, tsub * 128:tsub * 128 + 128],
                                    wvt[dmk][:],
                                    start=(dmk == 0), stop=(dmk == NDMK - 1))
                            nc.scalar.copy(v_sb[:, tt, :], ps_v[:])

                # ---------------- phase 2: attention ----------------
                with tc.tile_pool(name="es", bufs=2) as espool, \
                     tc.tile_pool(name="bc", bufs=3) as bcpool, \
                     tc.tile_pool(name="psS", bufs=2, space="PSUM") as psS, \
                     tc.tile_pool(name="psO", bufs=2, space="PSUM") as psO, \
                     tc.tile_pool(name="psN", bufs=2, space="PSUM") as psN:
                    # attention: per (head, 512-query block)
                    for h in range(HPC):
                        for J in range(NQB):
                            qs_ = slice(J * 512, J * 512 + 512)
                            nkt = 4 * J + 4          # causal k-tiles
                            ps_o = psO.tile([128, 512], F32, tag="po")
                            ps_s = psN.tile([1, 512], F32, tag="pn")
                            ngrp = nkt // 2
                            for g in range(ngrp):
                                ps_sc = psS.tile([128, 1024], F32, tag="sc")
                                es = espool.tile([128, 1024], F32R, tag="es")
                                for t2 in range(2):
                                    kt = 2 * g + t2
                                    sl = slice(t2 * 512, t2 * 512 + 512)
                                    diag = kt >= 4 * J
                                    nc.tensor.matmul(
                                        ps_sc[:, sl],
                                        kcatT[:, h, kt * 128:kt * 128 + 128],
                                        qcatT[:, h, qs_],
                                        start=True,
                                        stop=(MASK_DVE or not diag))
                                    if diag:
                                        t = kt - 4 * J
                                        j0 = 384 - 128 * t
                                        if MASK_DVE:
                                            nc.vector.tensor_add(
                                                ps_sc[:, sl], ps_sc[:, sl],
                                                masksB[:, j0:j0 + 512]
                                                .bitcast(F32))
                                        else:
                                            nc.tensor.matmul(
                                                ps_sc[:, sl], ident[:],
                                                masksB[:, j0:j0 + 512],
                                                start=False, stop=True)
                                nc.scalar.activation(es[:], ps_sc[:], Exp)
                                for t2 in range(2):
                                    kt = 2 * g + t2
                                    sl = slice(t2 * 512, t2 * 512 + 512)
                                    nc.tensor.matmul(
                                        ps_o[:],
                                        v_sb[:, kt, h * 128:h * 128 + 128],
                                        es[:, sl],
                                        start=(kt == 0), stop=(kt == nkt - 1))
                                    nc.tensor.matmul(
                                        ps_s[:], ones[:], es[:, sl],
                                        start=(kt == 0), stop=(kt == nkt - 1))
                            nc.vector.tensor_copy(attn_outT[:, h, qs_],
                                                  ps_o[:])
                            # normalize: broadcast sums across partitions,
                            # fast reciprocal, scale the block in place
                            sums_sb = bcpool.tile([1, 512], F32, tag="ssb")
                            nc.vector.tensor_copy(sums_sb[:], ps_s[:])
                            bc = bcpool.tile([128, 512], F32, tag="bc")
                            nc.gpsimd.partition_broadcast(bc[:], sums_sb[:])
                            bcr = bcpool.tile([128, 512], F32, tag="bcr")
                            nc.vector.reciprocal_approx_fast(bcr[:], bc[:])
                            nc.vector.tensor_mul(attn_outT[:, h, qs_],
                                                 attn_outT[:, h, qs_], bcr[:])

            # ---------------- phase 3: output projection ----------------
            with tc.tile_pool(name="wo", bufs=4) as wopool, \
                 tc.tile_pool(name="ost", bufs=3) as ostp, \
                 tc.tile_pool(name="psW", bufs=4, space="PSUM") as psW:
                for dmt in range(NDMK):
                    wo_t = wopool.tile([128, HPC, 128], F32R, tag="wo")
                    nc.sync.dma_start(out=wo_t[:], in_=wo_d[dmt, :, :, :])
                    for tch in range(NTCH):
                        ts_ = slice(tch * 512, tch * 512 + 512)
                        ps_w = psW.tile([128, 512], F32, tag="pw")
                        for h in range(HPC):
                            nc.tensor.matmul(
                                ps_w[:],
                                wo_t[:, h, :],
                                attn_outT[:, h, ts_],
                                start=(h == 0), stop=(h == HPC - 1))
                        o_sb = ostp.tile([128, 512], F32, tag="ost")
                        nc.scalar.copy(o_sb[:], ps_w[:])
                        nc.sync.dma_start(
                            out=out_d[dmt * 128:dmt * 128 + 128, ts_],
                            in_=o_sb[:])

    nc.compile()
    return nc


def _host_prep(x, wq_sem, wk_sem, wq_geo, wk_geo, wv, wo, gate_logit):
    """Build the 8 per-core input maps."""
    g = 1.0 / (1.0 + np.exp(-gate_logit.astype(np.float64)))  # [H]
    sc = 1.0 / np.sqrt(DS)

    half = DG // 2
    inv_freq = 1.0 / (10000.0 ** (np.arange(half, dtype=np.float64) / half))
    ang = np.arange(S, dtype=np.float64)[:, None] * inv_freq[None, :]  # [S, 32]
    cosT = np.cos(ang).T
    sinT = np.sin(ang).T
    cos2 = np.ascontiguousarray(
        np.concatenate([cosT, cosT], 0).astype(np.float16))          # [64, S]
    sins = np.ascontiguousarray(
        np.concatenate([-sinT, sinT], 0).astype(np.float16))         # [64, S]

    # sliding causal staircase: masks[kp, j] = 0 iff (j - 384) >= kp.
    # diag variant t uses window [384-128t : 896-128t].
    kp = np.arange(128)[:, None]
    j = np.arange(896)[None, :]
    masks = np.where(j - 384 >= kp, 0.0, MASK_VAL).astype(np.float32)
    ident = np.eye(128, dtype=np.float32)
    ones = np.ones((128, 1), dtype=np.float32)

    in_maps = []
    for c in range(NCORES):
        b, hg = divmod(c, HPC)
        heads = range(hg * HPC, hg * HPC + HPC)
        wq_cat = np.empty((DM, HPC * DH), dtype=np.float32)
        wk_cat = np.empty((DM, HPC * DH), dtype=np.float32)
        for i, h in enumerate(heads):
            gh = g[h]
            wq_cat[:, i * DH:i * DH + DS] = \
                wq_sem[:, h * DS:(h + 1) * DS] * np.float32(gh * sc)
            wq_cat[:, i * DH + DS:(i + 1) * DH] = \
                wq_geo[:, h * DG:(h + 1) * DG] * np.float32((1.0 - gh) * sc)
            wk_cat[:, i * DH:i * DH + DS] = wk_sem[:, h * DS:(h + 1) * DS]
            wk_cat[:, i * DH + DS:(i + 1) * DH] = wk_geo[:, h * DG:(h + 1) * DG]
        # pre-tile: wqk[fb, p, dmk, c] = w_cat[dmk*128+p, fb*128+c]
        wq_t = wq_cat.reshape(NDMK, 128, HPC, 128).transpose(2, 1, 0, 3)
        wk_t = wk_cat.reshape(NDMK, 128, HPC, 128).transpose(2, 1, 0, 3)
        wqk = np.ascontiguousarray(np.concatenate([wq_t, wk_t], 0))
        h0 = hg * HPC * DV
        wv_slice = wv[:, h0:h0 + HPC * DV]
        wv_t = np.ascontiguousarray(
            wv_slice.reshape(NDMK, 128, HPC * DV).transpose(1, 0, 2))
        wo_slice = wo[h0:h0 + HPC * DV, :]
        wo_t = np.ascontiguousarray(
            wo_slice.reshape(HPC, 128, NDMK, 128).transpose(2, 1, 0, 3))
        in_maps.append({
            "xT": np.ascontiguousarray(x[b].T).astype(PROJ_NP),
            "wqk": wqk.astype(PROJ_NP),
            "wv": wv_t.astype(PROJ_NP),
            "wo": wo_t,
            "cos2": cos2,
            "sins": sins,
            "masks": masks,
            "ident": ident,
            "ones": ones,
        })
    return in_maps


def _run(in_maps, **kw):
    if "nc" not in _CACHED:
        _CACHED["nc"] = _build()
    return run_bass_kernel_spmd(_CACHED["nc"], in_maps,
                                core_ids=list(range(NCORES)), **kw)


def kernel(x, wq_sem, wk_sem, wq_geo, wk_geo, wv, wo, gate_logit, **_kw):
    x = np.asarray(x, dtype=np.float32)
    wq_sem = np.asarray(wq_sem, dtype=np.float32)
    wk_sem = np.asarray(wk_sem, dtype=np.float32)
    wq_geo = np.asarray(wq_geo, dtype=np.float32)
    wk_geo = np.asarray(wk_geo, dtype=np.float32)
    wv = np.asarray(wv, dtype=np.float32)
    wo = np.asarray(wo, dtype=np.float32)
    gate_logit = np.asarray(gate_logit, dtype=np.float32)

    in_maps = _host_prep(x, wq_sem, wk_sem, wq_geo, wk_geo, wv, wo, gate_logit)
    res = _run(in_maps)
    out = np.zeros((B, S, DM), dtype=np.float32)
    for c in range(NCORES):
        out[c // HPC] += res.results[c]["out"].T
    return out



# revision 2
# speedup vs baseline: 12.1163x; 12.1163x over previous
"""Trainium2 Bass kernel for the gated two-path (semantic+RoPE-geometric) causal
attention layer.  8-core sharding: 2 heads x BOTH batches per core, with
on-device collectives so every unique input byte crosses PJRT exactly once.

Reference computation (B=2, S=2048, D_MODEL=2048, H=16, DS=DG=64, DV=128):
  qs=x@wq_sem, ks=x@wk_sem, qg=rope(x@wq_geo), kg=rope(x@wk_geo), v=x@wv
  scores = g*qs.ks/8 + (1-g)*qg.kg/8 ; causal softmax ; out=(attn@v)@wo

Data distribution (everything bf16 on the wire):
  - x is sharded 8 ways by dm-rows per 512-token chunk and AllGathered
    on-device (8 chunk-AGs so phase 1 streams as chunks land)
  - wq/wk/wv slices for the core's 2 heads ship directly (distinct per core)
  - wo + trig tables + causal mask are identical everywhere: sharded 1/8
    per core and AllGathered (trig ships as f16 bits inside the bf16 blob)
  - after attention, an 8-rank AllToAll reshards from (2 heads, all tokens)
    to (all 16 heads, my 512-token eighth); each core then computes the
    full output projection for its eighth -> out [DM, 512] bf16

Per-core compute (mirrors the previous 4-head kernel; a "slot" is now a
(batch, head) pair instead of a head):
  - host folds sigmoid(gate)/sqrt(d) into wq and concatenates [sem|geo]
    per head so each head's QK^T is one K=128 contraction
  - projections: qcatT/kcatT [128, slot, S] via lhsT=weight tiles,
    rhs=xT chunks from the AG buffers; rope fused per 512-token slice
  - scores^T per k-tile; causal staircase mask added on DVE into PSUM
  - exp on ScalarE (no max subtraction; |scores| <~ 8 << 88), AV +
    ones-matmul denominators in PSUM, gpsimd broadcast + fast reciprocal
  - output projection contracts all 16 heads for the core's token eighth
"""

import sys

sys.path.insert(0, "/opt/trn_rl_repo")

import ml_dtypes
import numpy as np

import concourse.mybir as mybir
import concourse.tile as tile
from concourse import bacc
from concourse.bass_utils import run_bass_kernel_spmd

F32 = mybir.dt.float32
F32R = mybir.dt.float32r
F16 = mybir.dt.float16
BF16 = mybir.dt.bfloat16
BF16_NP = ml_dtypes.bfloat16

B, S, DM = 2, 2048, 2048
H, DS, DG, DV = 16, 64, 64, 128
HPC = 2                      # heads per core
NCORES = 8
DH = DS + DG                 # 128, concat [sem|geo] per head
NSLOT = 4                    # (batch, local head) pairs per core
NKT = S // 128               # 16 key tiles per batch
NQB = S // 512               # 4 query blocks per batch
NGCH = 8                     # global 512-token chunks (2 batches x 4)
NDMK = DM // 128             # 16 contraction tiles
MASK_VAL = -10000.0
G8 = [[0, 1, 2, 3, 4, 5, 6, 7]]

_CACHED = {}


def _build(repeat=1):
    nc = bacc.Bacc("TRN2", target_bir_lowering=False, debug=False,
                   num_devices=NCORES)

    # per-core inputs (see _host_prep for layouts)
    xin_d = nc.dram_tensor("xin", [NGCH, 256, 512], BF16,
                           kind="ExternalInput").ap()
    wqk_d = nc.dram_tensor("wqk", [2 * HPC, 128, NDMK, 128], BF16,
                           kind="ExternalInput").ap()
    wv_d = nc.dram_tensor("wv", [128, NDMK, 256], BF16,
                          kind="ExternalInput").ap()
    cin_d = nc.dram_tensor("cin", [288, 2048], BF16,
                           kind="ExternalInput").ap()
    out_d = nc.dram_tensor("out", [DM, 512], BF16, kind="ExternalOutput").ap()

    Exp = mybir.ActivationFunctionType.Exp

    with tile.TileContext(nc) as tc:
      for _rep in range(repeat):
        with tc.tile_pool(name="coll", bufs=1, space="DRAM") as dpool, \
             tc.tile_pool(name="consts", bufs=1) as cpool:
            # ---- collective staging: bounce -> AllGather ----
            b_tm = dpool.tile([32, 2048], BF16)
            b_wo = dpool.tile([256, 2048], BF16)
            b_x = dpool.tile([NGCH, 256, 512], BF16)
            tmag = dpool.tile([256, 2048], BF16, addr_space="Shared")
            woag = dpool.tile([2048, 2048], BF16, addr_space="Shared")
            xag = [dpool.tile([2048, 512], BF16, addr_space="Shared",
                              name=f"xag{g}_r{_rep}")
                   for g in range(NGCH)]
            a2a_in = dpool.tile([8, 128, HPC, 512], BF16)
            a2a_out = dpool.tile([8, 128, HPC, 512], BF16)

            nc.gpsimd.dma_start(b_tm[:], cin_d[256:288, :])
            nc.gpsimd.collective_compute(
                "AllGather", mybir.AluOpType.bypass, replica_groups=G8,
                ins=[b_tm[:].opt()], outs=[tmag[:].opt()])
            for g in range(NGCH):
                nc.gpsimd.dma_start(b_x[g], xin_d[g, :, :])
                nc.gpsimd.collective_compute(
                    "AllGather", mybir.AluOpType.bypass, replica_groups=G8,
                    ins=[b_x[g].opt()], outs=[xag[g][:].opt()])
            nc.gpsimd.dma_start(b_wo[:], cin_d[0:256, :])
            nc.gpsimd.collective_compute(
                "AllGather", mybir.AluOpType.bypass, replica_groups=G8,
                ins=[b_wo[:].opt()], outs=[woag[:].opt()])

            # ---- constants to SBUF ----
            ones = cpool.tile([128, 1], BF16)
            nc.gpsimd.memset(ones[:], 1.0)
            # trig tables at base partition 64 so two-input DVE rope ops
            # share their operands' base partition
            trig = cpool.tile([128, 2, S], F16)
            masksB = cpool.tile([128, 896], BF16)
            nc.sync.dma_start(out=trig[64:128, 0, :],
                              in_=tmag[0:64, :].bitcast(F16))
            nc.sync.dma_start(out=trig[64:128, 1, :],
                              in_=tmag[64:128, :].bitcast(F16))
            nc.sync.dma_start(out=masksB[:], in_=tmag[128:256, 0:896])

            with tc.tile_pool(name="persist", bufs=1) as ppool:
                qcatT = ppool.tile([128, NSLOT, S], BF16)
                kcatT = ppool.tile([128, NSLOT, S], BF16)
                v_sb = ppool.tile([128, NKT, 512], BF16)

                # -------- phase 1: projections (+ rope fused) --------
                with tc.tile_pool(name="xt", bufs=NDMK + 2) as xtp, \
                     tc.tile_pool(name="wcol", bufs=5) as wcp, \
                     tc.tile_pool(name="wvst", bufs=1) as wvp, \
                     tc.tile_pool(name="rot", bufs=2) as rpool, \
                     tc.tile_pool(name="psA", bufs=4, space="PSUM") as psA:

                    wcs = []
                    for fb in range(2 * HPC):
                        wc = wcp.tile([128, NDMK, 128], BF16, tag="wc")
                        nc.sync.dma_start(out=wc[:], in_=wqk_d[fb, :, :, :])
                        wcs.append(wc)
                    wvt = wvp.tile([128, NDMK, 256], BF16)
                    nc.sync.dma_start(out=wvt[:], in_=wv_d[:, :, :])

                    for gch in range(NGCH):
                        b, tc_ = divmod(gch, 4)
                        ts_ = slice(tc_ * 512, tc_ * 512 + 512)
                        xts = []
                        for dmk in range(NDMK):
                            xt_t = xtp.tile([128, 512], BF16, tag="xt")
                            nc.sync.dma_start(
                                out=xt_t[:],
                                in_=xag[gch][dmk * 128:dmk * 128 + 128, :])
                            xts.append(xt_t)
                        # qcat / kcat columns: 4 feature blocks of 128
                        for fb in range(2 * HPC):
                            h = fb % HPC
                            slot = b * HPC + h
                            ps_t = psA.tile([128, 512], F32, tag="ps")
                            for dmk in range(NDMK):
                                nc.tensor.matmul(
                                    ps_t[:],
                                    wcs[fb][:, dmk, :],
                                    xts[dmk][:],
                                    start=(dmk == 0), stop=(dmk == NDMK - 1))
                            X = qcatT if fb < HPC else kcatT
                            nc.scalar.copy(X[:, slot, ts_], ps_t[:])
                            # rope this 512-token slice of the geo half
                            rot = rpool.tile([128, 512], BF16, tag="rot")
                            nc.gpsimd.tensor_copy(rot[64:96, :],
                                                  X[96:128, slot, ts_])
                            nc.gpsimd.tensor_copy(rot[96:128, :],
                                                  X[64:96, slot, ts_])
                            nc.vector.tensor_mul(rot[64:128, :],
                                                 rot[64:128, :],
                                                 trig[64:128, 1, ts_])
                            nc.vector.tensor_mul(X[64:128, slot, ts_],
                                                 X[64:128, slot, ts_],
                                                 trig[64:128, 0, ts_])
                            nc.vector.tensor_add(X[64:128, slot, ts_],
                                                 X[64:128, slot, ts_],
                                                 rot[64:128, :])
                        # v: natural layout [token, 2*dv], 4 token sub-tiles
                        for tsub in range(4):
                            tt = tc_ * 4 + tsub
                            ps_v = psA.tile([128, 256], F32, tag="psv")
                            for dmk in range(NDMK):
                                nc.tensor.matmul(
                                    ps_v[:],
                                    xts[dmk][:, tsub * 128:tsub * 128 + 128],
                                    wvt[:, dmk, :],
                                    start=(dmk == 0), stop=(dmk == NDMK - 1))
                            nc.scalar.copy(
                                v_sb[:, tt, b * 256:b * 256 + 256], ps_v[:])

                # -------- phase 2: attention --------
                with tc.tile_pool(name="es", bufs=2) as espool, \
                     tc.tile_pool(name="bc", bufs=3) as bcpool, \
                     tc.tile_pool(name="stg", bufs=2) as stpool, \
                     tc.tile_pool(name="psS", bufs=2, space="PSUM") as psS, \
                     tc.tile_pool(name="psO", bufs=2, space="PSUM") as psO, \
                     tc.tile_pool(name="psN", bufs=2, space="PSUM") as psN:
                    for slot in range(NSLOT):
                        b, h = divmod(slot, HPC)
                        for J in range(NQB):
                            qs_ = slice(J * 512, J * 512 + 512)
                            nkt = 4 * J + 4          # causal k-tiles
                            ps_o = psO.tile([128, 512], F32, tag="po")
                            ps_s = psN.tile([1, 512], F32, tag="pn")
                            ngrp = nkt // 2
                            for g in range(ngrp):
                                ps_sc = psS.tile([128, 1024], F32, tag="sc")
                                es = espool.tile([128, 1024], BF16, tag="es")
                                for t2 in range(2):
                                    kt = 2 * g + t2
                                    sl = slice(t2 * 512, t2 * 512 + 512)
                                    diag = kt >= 4 * J
                                    nc.tensor.matmul(
                                        ps_sc[:, sl],
                                        kcatT[:, slot,
                                              kt * 128:kt * 128 + 128],
                                        qcatT[:, slot, qs_],
                                        start=True, stop=True)
                                    if diag:
                                        t = kt - 4 * J
                                        j0 = 384 - 128 * t
                                        nc.vector.tensor_add(
                                            ps_sc[:, sl], ps_sc[:, sl],
                                            masksB[:, j0:j0 + 512])
                                nc.scalar.activation(es[:], ps_sc[:], Exp)
                                for t2 in range(2):
                                    kt = 2 * g + t2
                                    sl = slice(t2 * 512, t2 * 512 + 512)
                                    nc.tensor.matmul(
                                        ps_o[:],
                                        v_sb[:, kt,
                                             slot * 128:slot * 128 + 128],
                                        es[:, sl],
                                        start=(kt == 0), stop=(kt == nkt - 1))
                                    nc.tensor.matmul(
                                        ps_s[:], ones[:], es[:, sl],
                                        start=(kt == 0), stop=(kt == nkt - 1))
                            # normalize: broadcast sums across partitions,
                            # fast reciprocal, scale + downcast into staging
                            sums_sb = bcpool.tile([1, 512], F32, tag="ssb")
                            nc.vector.tensor_copy(sums_sb[:], ps_s[:])
                            bc = bcpool.tile([128, 512], F32, tag="bc")
                            nc.gpsimd.partition_broadcast(bc[:], sums_sb[:])
                            bcr = bcpool.tile([128, 512], F32, tag="bcr")
                            nc.vector.reciprocal_approx_fast(bcr[:], bc[:])
                            stg = stpool.tile([128, 512], BF16, tag="stg")
                            nc.vector.tensor_mul(stg[:], ps_o[:], bcr[:])
                            nc.sync.dma_start(
                                out=a2a_in[b * 4 + J, :, h, :], in_=stg[:])

            # -------- reshard: (2 heads, all tokens) -> (16 heads, eighth)
            nc.gpsimd.collective_compute(
                "AllToAll", mybir.AluOpType.bypass, replica_groups=G8,
                ins=[a2a_in[:].opt()], outs=[a2a_out[:].opt()])

            # -------- phase 3: output projection for my token eighth ------
            with tc.tile_pool(name="att", bufs=1) as apool, \
                 tc.tile_pool(name="wo", bufs=4) as wopool, \
                 tc.tile_pool(name="ost", bufs=3) as ostp, \
                 tc.tile_pool(name="psW", bufs=4, space="PSUM") as psW:
                att = apool.tile([128, H, 512], BF16)
                for i in range(8):
                    for hl in range(HPC):
                        nc.sync.dma_start(out=att[:, i * HPC + hl, :],
                                          in_=a2a_out[i, :, hl, :])
                for dmt in range(NDMK):
                    wo_t = wopool.tile([128, H, 128], BF16, tag="wo")
                    nc.sync.dma_start(
                        out=wo_t[:],
                        in_=woag[dmt * 128:dmt * 128 + 128, :])
                    ps_w = psW.tile([128, 512], F32, tag="pw")
                    for h in range(H):
                        nc.tensor.matmul(
                            ps_w[:],
                            wo_t[:, h, :],
                            att[:, h, :],
                            start=(h == 0), stop=(h == H - 1))
                    o_sb = ostp.tile([128, 512], BF16, tag="ost")
                    nc.scalar.copy(o_sb[:], ps_w[:])
                    nc.sync.dma_start(
                        out=out_d[dmt * 128:dmt * 128 + 128, :], in_=o_sb[:])

    nc.compile()
    return nc


def _host_prep(x, wq_sem, wk_sem, wq_geo, wk_geo, wv, wo, gate_logit):
    """Build the 8 per-core input maps (all bf16 on the wire)."""
    g = 1.0 / (1.0 + np.exp(-gate_logit.astype(np.float64)))  # [H]
    sc = 1.0 / np.sqrt(DS)

    half = DG // 2
    inv_freq = 1.0 / (10000.0 ** (np.arange(half, dtype=np.float64) / half))
    ang = np.arange(S, dtype=np.float64)[:, None] * inv_freq[None, :]
    cosT = np.cos(ang).T
    sinT = np.sin(ang).T
    cos2 = np.concatenate([cosT, cosT], 0).astype(np.float16)       # [64, S]
    sins = np.concatenate([-sinT, sinT], 0).astype(np.float16)      # [64, S]
    # f16 bits carried through the bf16 AllGather blob
    trig_rows = np.ascontiguousarray(
        np.concatenate([cos2, sins], 0)).view(np.uint16).view(BF16_NP)

    # sliding causal staircase: masks[kp, j] = 0 iff (j - 384) >= kp.
    kp = np.arange(128)[:, None]
    j = np.arange(896)[None, :]
    masks = np.where(j - 384 >= kp, 0.0, MASK_VAL).astype(BF16_NP)
    masks_rows = np.zeros((128, 2048), dtype=BF16_NP)
    masks_rows[:, :896] = masks

    # wo tiled: cag_wo[dmt*128+p, h*128+c] = wo[h*128+p, dmt*128+c]
    wo_rows = np.ascontiguousarray(
        wo.reshape(H, 128, NDMK, 128).transpose(2, 1, 0, 3)
        .reshape(DM, DM)).astype(BF16_NP)
    tm_rows = np.concatenate([trig_rows, masks_rows], 0)    # [256, 2048]

    xt_b = [np.ascontiguousarray(x[b].T).astype(BF16_NP) for b in range(B)]

    in_maps = []
    for c in range(NCORES):
        heads = [2 * c, 2 * c + 1]
        # wqk: fb 0,1 = q-cat for local heads; fb 2,3 = k-cat
        wqk = np.empty((2 * HPC, 128, NDMK, 128), dtype=np.float32)
        for i, h in enumerate(heads):
            gh = g[h]
            wq_cat = np.empty((DM, DH), dtype=np.float32)
            wq_cat[:, :DS] = wq_sem[:, h * DS:(h + 1) * DS] * \
                np.float32(gh * sc)
            wq_cat[:, DS:] = wq_geo[:, h * DG:(h + 1) * DG] * \
                np.float32((1.0 - gh) * sc)
            wk_cat = np.concatenate(
                [wk_sem[:, h * DS:(h + 1) * DS],
                 wk_geo[:, h * DG:(h + 1) * DG]], 1)
            # [p, dmk, c] = w[dmk*128+p, c]
            wqk[i] = wq_cat.reshape(NDMK, 128, DH).transpose(1, 0, 2)
            wqk[2 + i] = wk_cat.reshape(NDMK, 128, DH).transpose(1, 0, 2)
        # wv2[p, dmk, h*128+cc] = wv[dmk*128+p, (2c+h)*128+cc]
        wv_slice = wv[:, 2 * c * DV:(2 * c + 2) * DV]       # [DM, 256]
        wv_t = wv_slice.reshape(NDMK, 128, 256).transpose(1, 0, 2)
        # x: my 1/8 of dm-rows for each global 512-token chunk
        xin = np.empty((NGCH, 256, 512), dtype=BF16_NP)
        for gch in range(NGCH):
            b, tcc = divmod(gch, 4)
            xin[gch] = xt_b[b][256 * c:256 * c + 256,
                               tcc * 512:tcc * 512 + 512]
        cin = np.concatenate(
            [wo_rows[256 * c:256 * c + 256], tm_rows[32 * c:32 * c + 32]], 0)
        in_maps.append({
            "xin": xin,
            "wqk": np.ascontiguousarray(wqk).astype(BF16_NP),
            "wv": np.ascontiguousarray(wv_t).astype(BF16_NP),
            "cin": np.ascontiguousarray(cin),
        })
    return in_maps


def _run(in_maps, **kw):
    if "nc" not in _CACHED:
        _CACHED["nc"] = _build()
    return run_bass_kernel_spmd(_CACHED["nc"], in_maps,
                                core_ids=list(range(NCORES)), **kw)


def _assemble(results):
    out = np.empty((B, S, DM), dtype=np.float32)
    for c in range(NCORES):
        b, q = divmod(c, 4)
        out[b, q * 512:(q + 1) * 512, :] = \
            results[c]["out"].astype(np.float32).T
    return out


def kernel(x, wq_sem, wk_sem, wq_geo, wk_geo, wv, wo, gate_logit, **_kw):
    x = np.asarray(x, dtype=np.float32)
    wq_sem = np.asarray(wq_sem, dtype=np.float32)
    wk_sem = np.asarray(wk_sem, dtype=np.float32)
    wq_geo = np.asarray(wq_geo, dtype=np.float32)
    wk_geo = np.asarray(wk_geo, dtype=np.float32)
    wv = np.asarray(wv, dtype=np.float32)
    wo = np.asarray(wo, dtype=np.float32)
    gate_logit = np.asarray(gate_logit, dtype=np.float32)

    in_maps = _host_prep(x, wq_sem, wk_sem, wq_geo, wk_geo, wv, wo, gate_logit)
    res = _run(in_maps)
    return _assemble(res.results)
